# revision 12
# baseline (speedup 1.0000x reference)
"""Trainium2 Bass kernel for nn_MiTransformer (moe_routing).

Sharding: data-parallel over batch B=16 across 8 cores (2 batch items per
core), weights replicated. On-chip, token activations are kept TRANSPOSED
([D, tokens]) so every projection is a natural PE matmul with the
contraction on partitions; attention heads (dh=32) are packed 4-per-128
partition chunk so score/AV matmuls run concurrently via PE row/col
tiling. Softmax denominators come from an all-ones matmul (partition-dim
column sum + broadcast in one PE pass). The MoE (E=8, FF=16) is computed
densely with experts padded to 32 lanes; top-2 routing becomes a masked
weight combine folded into the PE down-projection accumulation. All
matmuls use float32r (full-rate fp32 PE mode). ScalarE uses only
Exp/Ln/Identity/Square so one activation table set serves the kernel.

Self-contained: hardcodes all shapes from the problem spec.
"""
import os
import numpy as np
from contextlib import ExitStack

import concourse.bass as bass
import concourse.tile as tile
import concourse.mybir as mybir
from concourse import bacc
from concourse.bass_utils import run_bass_kernel_spmd

AF = mybir.ActivationFunctionType
OP = mybir.AluOpType
F32 = mybir.dt.float32
F32R = mybir.dt.float32r

B, NV, SL, D, H, E, FF, TK, NL, NI = 16, 512, 1024, 512, 16, 8, 16, 2, 2, 64
EPS = 1e-5
P = 128
KC = D // P          # 4 d-chunks
NT = NV              # tokens per batch item
NB = B // 8          # batch items per core
LC = SL // P         # 8 time chunks
DH = D // H          # 32
G = 4                # head groups (4 heads x 32 rows)
SCALE = 1.0 / float(np.sqrt(DH))
N_LAYERS = int(os.environ.get("KERNEL_NL", str(NL)))
X = mybir.AxisListType.X


def R(ap):
    return ap.bitcast(F32R)


def build_program(n_layers=N_LAYERS):
    nc = bacc.Bacc("TRN2", target_bir_lowering=False, debug=False,
                   enable_asserts=False, num_devices=8)

    def di(name, shape, dt=F32R):
        return nc.dram_tensor(name, list(shape), dt, kind="ExternalInput").ap()

    x_d = di("x", (NB, SL, NV))
    ins_d = di("ins", (NI, D))
    wembT_d = di("wembT", (SL, D))
    wsum_d = di("wsum_r", (2, D))
    wqT_d = di("wqT", (NL, D, D))
    wkT_d = di("wkT", (NL, D, D))
    wvT_d = di("wvT", (NL, D, D))
    woT_d = di("woT", (NL, D, D))
    bq_d = di("bq", (NL, D), F32)
    bk_d = di("bk", (NL, D), F32)
    boe_d = di("bo_eff", (NL, D), F32)
    ln1g_d = di("ln1g", (NL, D), F32)
    ln1b_d = di("ln1b", (NL, D), F32)
    ln2g_d = di("ln2g", (NL, D), F32)
    ln2b_d = di("ln2b", (NL, D), F32)
    gateT_d = di("gateT", (NL, D, P))
    egT_d = di("egT", (NL, 2, D, P))
    euT_d = di("euT", (NL, 2, D, P))
    edT_d = di("edT", (NL, 2, P, D))
    gamTw_d = di("gamTw", (NL, D, P))
    betaT_d = di("betaT", (NL, E, D, D))
    encg_d = di("encg", (D,), F32)
    encb_d = di("encb", (D,), F32)
    projT_d = di("projT", (D, D))
    projb_d = di("projb_r", (2, D))
    revw_d = di("revw_r", (2, D))
    revb_d = di("revb_r", (2, D))
    ident_d = di("ident", (P, P))
    onesm_d = di("onesm", (P, P))
    indpad_d = di("indpad", (2, E, P))
    identf_d = di("identf", (P, P), F32)
    zrow_d = di("zrow", (1, D))

    out_d = nc.dram_tensor("out", [NB, NV, D], F32, kind="ExternalOutput").ap()

    with tile.TileContext(nc) as tc, ExitStack() as ctx:
        cst = ctx.enter_context(tc.tile_pool(name="cst", bufs=1))
        wp = ctx.enter_context(tc.tile_pool(name="wp", bufs=1))
        stm = ctx.enter_context(tc.tile_pool(name="stm", bufs=3))
        act = ctx.enter_context(tc.tile_pool(name="act", bufs=1))
        sm = ctx.enter_context(tc.tile_pool(name="sm", bufs=1))
        sm2 = ctx.enter_context(tc.tile_pool(name="sm2", bufs=2))
        xp = ctx.enter_context(tc.tile_pool(name="xp", bufs=1))
        ep = ctx.enter_context(tc.tile_pool(name="ep", bufs=1))
        ps = ctx.enter_context(tc.tile_pool(name="ps", bufs=6, space="PSUM"))
        ps2 = ctx.enter_context(tc.tile_pool(name="ps2", bufs=2, space="PSUM"))

        def big_ps():
            return ps.tile([P, NT], F32, tag="big", name="bigps")

        def small_ps(shape):
            return ps2.tile(list(shape), F32, tag="small", name="smallps")

        # ---- constants --------------------------------------------------
        ident = cst.tile([P, P], F32R, tag="ident")
        nc.sync.dma_start(ident[:], ident_d)
        ones = cst.tile([P, P], F32R, tag="ones")
        nc.sync.dma_start(ones[:], onesm_d)
        ins_sb = cst.tile([NI, D], F32R, tag="ins")
        nc.sync.dma_start(ins_sb[:], ins_d)
        indp = cst.tile([E, 2 * P], F32R, tag="indp")
        nc.sync.dma_start(indp[:, 0:P], indpad_d[0])
        nc.sync.dma_start(indp[:, P:2 * P], indpad_d[1])
        revw_r = cst.tile([2, D], F32R, tag="revw_r")
        nc.sync.dma_start(revw_r[:], revw_d)
        revb_r = cst.tile([2, D], F32R, tag="revb_r")
        nc.sync.dma_start(revb_r[:], revb_d)
        wsum_r = cst.tile([2, D], F32R, tag="wsum_r")
        nc.sync.dma_start(wsum_r[:], wsum_d)
        projb_r = cst.tile([2, D], F32R, tag="projb_r")
        nc.sync.dma_start(projb_r[:], projb_d)
        eps_c = cst.tile([P, 1], F32, tag="eps_c")
        nc.vector.memset(eps_c[:], EPS)
        identf = cst.tile([P, P], F32, tag="identf")
        nc.sync.dma_start(identf[:], identf_d)
        onesb = cst.tile([P, 32], mybir.dt.bfloat16, tag="onesb")
        nc.vector.memset(onesb[:], 1.0)

        def row_to_col(row_ap, tag, dt=F32):
            col = cst.tile([P, KC], dt, tag=tag)
            for k in range(KC):
                pt = small_ps((P, 2))
                nc.tensor.matmul(pt[:], R(row_ap[0:2, k * P:(k + 1) * P]),
                                 R(ident[0:2, 0:2]), start=True, stop=True)
                nc.vector.tensor_copy(col[:, k:k + 1], pt[:, 0:1])
            return col

        # ---- instruction-token mean, gamma, beta ------------------------
        ps_ir = big_ps()
        nc.tensor.matmul(ps_ir[:], R(ones[0:NI, :]), R(ins_sb[:]),
                         start=True, stop=True)
        insm_row = cst.tile([2, D], F32R, tag="insm_row")
        nc.sync.dma_start(insm_row[1:2, :], zrow_d)
        nc.vector.tensor_scalar_mul(insm_row[0:1, :], ps_ir[0:1, :], 1.0 / NI)
        insm_c = row_to_col(insm_row, "insm_c", F32R)
        insmB = cst.tile([P, KC, P], F32R, tag="insmB")
        for k in range(KC):
            nc.vector.tensor_copy(insmB[:, k, :],
                                  insm_c[:, k:k + 1].to_broadcast((P, P)))

        gamT = []
        for l in range(NL):
            gw = cst.tile([P, KC, P], F32R, tag=f"gamw{l}")
            nc.sync.dma_start(gw[:], gamTw_d[l].rearrange("(k p) e -> p k e", p=P))
            pg = small_ps((P, 2))
            for k in range(KC):
                nc.tensor.matmul(pg[:], R(gw[:, k, :]), R(insmB[:, k, 0:2]),
                                 start=(k == 0), stop=(k == KC - 1))
            gt = cst.tile([E, 1], F32, tag=f"gamT{l}")
            nc.vector.tensor_scalar_mul(gt[:], pg[0:E, 0:1], 1.0 / NI)
            gamT.append(gt)

        betS = [cst.tile([E, D], F32R, tag=f"betS{l}", name=f"betS{l}") for l in range(NL)]

        def emit_bet(l):
            for e in range(E):
                pb = big_ps()
                for k in range(KC):
                    bt = stm.tile([P, D], F32R, tag="betaT")
                    nc.sync.dma_start(bt[:], betaT_d[l, e, k * P:(k + 1) * P, :])
                    nc.tensor.matmul(pb[:], R(insmB[:, k, :]), R(bt[:]),
                                     start=(k == 0), stop=(k == KC - 1))
                nc.vector.tensor_scalar_mul(betS[l][e:e + 1, :], pb[0:1, :],
                                            1.0 / NI)

        # ---- embedding + RevIN stats ------------------------------------
        encT = [None, None]
        mean_rows, std_rows = [], []
        for b in range(NB):
            pe = [big_ps() for _ in range(KC)]
            ps_s1 = big_ps()
            ps_s2 = big_ps()
            for c in range(LC):
                xc = stm.tile([P, NV], F32R, tag="xc")
                nc.sync.dma_start(xc[:], x_d[b, c * P:(c + 1) * P, :])
                wec = stm.tile([P, D], F32R, tag="wec")
                nc.sync.dma_start(wec[:], wembT_d[c * P:(c + 1) * P, :])
                xq = stm.tile([P, NV], F32R, tag="xq")
                nc.scalar.activation(xq[:], xc[:], AF.Square)
                nc.tensor.matmul(ps_s1[:], R(ones[:]), R(xc[:]),
                                 start=(c == 0), stop=(c == LC - 1))
                nc.tensor.matmul(ps_s2[:], R(ones[:]), R(xq[:]),
                                 start=(c == 0), stop=(c == LC - 1))
                for i in range(KC):
                    nc.tensor.matmul(pe[i][:], R(wec[:, i * P:(i + 1) * P]),
                                     R(xc[:]), start=(c == 0), stop=False)
            mean_r = cst.tile([2, NV], F32R, tag=f"mean_r{b}")
            nc.sync.dma_start(mean_r[1:2, :], zrow_d)
            nc.vector.tensor_scalar_mul(mean_r[0:1, :], ps_s1[0:1, :], 1.0 / SL)
            msq = sm.tile([1, NV], F32, tag="msq")
            nc.vector.tensor_mul(msq[:], mean_r[0:1, :], mean_r[0:1, :])
            var_r = sm.tile([1, NV], F32, tag="var_r")
            nc.vector.scalar_tensor_tensor(var_r[:], ps_s2[0:1, :], 1.0 / SL,
                                           msq[:], OP.mult, OP.subtract)
            lnv = sm.tile([1, NV], F32, tag="lnv")
            nc.scalar.activation(lnv[:], var_r[:], AF.Ln, bias=eps_c[0:1, :])
            std_r = cst.tile([2, NV], F32R, tag=f"std_r{b}")
            nc.sync.dma_start(std_r[1:2, :], zrow_d)
            nc.scalar.activation(std_r[0:1, :], lnv[:], AF.Exp, scale=0.5)
            rstd_r = sm.tile([1, NV], F32, tag="rstd_r")
            nc.scalar.activation(rstd_r[:], lnv[:], AF.Exp, scale=-0.5)
            mean_rows.append(mean_r)
            std_rows.append(std_r)
            a_r = sm.tile([2, NV], F32R, tag="a_r")
            nc.sync.dma_start(a_r[1:2, :], zrow_d)
            nc.vector.tensor_mul(a_r[0:1, :], revw_r[0:1, :], rstd_r[:])
            irw = sm.tile([1, NV], F32, tag="irw")
            nc.vector.reciprocal(irw[:], revw_r[0:1, :])
            coa = sm.tile([2, NV], F32R, tag="coa")
            nc.sync.dma_start(coa[1:2, :], zrow_d)
            nc.vector.tensor_mul(coa[0:1, :], revb_r[0:1, :], std_r[0:1, :])
            nc.vector.tensor_mul(coa[0:1, :], coa[0:1, :], irw[:])
            nc.vector.tensor_sub(coa[0:1, :], coa[0:1, :], mean_r[0:1, :])
            p_aB = big_ps()
            nc.tensor.matmul(p_aB[:], R(ones[0:2, :]), R(a_r[0:2, :]),
                             start=True, stop=True)
            aB = sm.tile([P, NV], F32, tag="aB")
            nc.vector.tensor_copy(aB[:], p_aB[:])
            eT = act.tile([P, KC, NT], F32R, tag=f"encT{b}")
            for i in range(KC):
                nc.tensor.matmul(pe[i][:], R(wsum_r[0:2, i * P:(i + 1) * P]),
                                 R(coa[0:2, :]), start=False, stop=True)
                nc.vector.tensor_mul(eT[:, i, :], pe[i][:], aB[:])
            encT[b] = eT

        # ---- layers -----------------------------------------------------
        def layernorm_T(sumT, g_cols, b_cols, out_tag):
            sq = xp.tile([P, KC, NT], F32R, tag="oT")
            nc.scalar.activation(sq[:], sumT[:], AF.Square)
            pm = big_ps()
            pv = big_ps()
            for k in range(KC):
                nc.tensor.matmul(pm[:], R(ones[:]), R(sumT[:, k, :]),
                                 start=(k == 0), stop=(k == KC - 1))
                nc.tensor.matmul(pv[:], R(ones[:]), R(sq[:, k, :]),
                                 start=(k == 0), stop=(k == KC - 1))
            mB = sm.tile([P, NT], F32, tag="ln_mB")
            nc.vector.tensor_scalar_mul(mB[:], pm[:], 1.0 / D)
            m2 = sm.tile([P, NT], F32, tag="ln_m2")
            nc.vector.tensor_mul(m2[:], mB[:], mB[:])
            varB = sm.tile([P, NT], F32, tag="ln_varB")
            nc.vector.scalar_tensor_tensor(varB[:], pv[:], 1.0 / D, m2[:],
                                           OP.mult, OP.subtract)
            lnv = sm.tile([P, NT], F32, tag="ln_lnv")
            nc.scalar.activation(lnv[:], varB[:], AF.Ln, bias=eps_c[:])
            rstdB = sm.tile([P, NT], F32, tag="ln_rstdB")
            nc.scalar.activation(rstdB[:], lnv[:], AF.Exp, scale=-0.5)
            outT = act.tile([P, KC, NT], F32R, tag=out_tag)
            for k in range(KC):
                tmp = sm2.tile([P, NT], F32, tag="ln_tmp")
                nc.vector.tensor_sub(tmp[:], sumT[:, k, :], mB[:])
                nc.vector.tensor_mul(tmp[:], tmp[:], rstdB[:])
                nc.vector.tensor_scalar(outT[:, k, :], tmp[:],
                                        g_cols[:, k:k + 1], b_cols[:, k:k + 1],
                                        OP.mult, OP.add)
            return outT

        def load_cols(dram_row, tag):
            t = wp.tile([P, KC], F32, tag=tag)
            nc.sync.dma_start(t[:], dram_row.rearrange("(k p) -> p k", p=P))
            return t

        for l in range(n_layers):
            wq = wp.tile([P, KC, D], F32R, tag="wq")
            nc.sync.dma_start(wq[:], wqT_d[l].rearrange("(k p) o -> p k o", p=P))
            wk = wp.tile([P, KC, D], F32R, tag="wk")
            nc.sync.dma_start(wk[:], wkT_d[l].rearrange("(k p) o -> p k o", p=P))
            wv = wp.tile([P, KC, D], F32R, tag="wv")
            nc.sync.dma_start(wv[:], wvT_d[l].rearrange("(k p) o -> p k o", p=P))
            wo = wp.tile([P, KC, D], F32R, tag="wo")
            nc.sync.dma_start(wo[:], woT_d[l].rearrange("(k p) o -> p k o", p=P))
            gte = wp.tile([P, KC, P], F32R, tag="gte")
            nc.sync.dma_start(gte[:], gateT_d[l].rearrange("(k p) e -> p k e", p=P))
            eg = wp.tile([P, KC, 2, P], F32R, tag="eg")
            nc.sync.dma_start(eg[:], egT_d[l].rearrange("h (k p) f -> p k h f", p=P))
            eu = wp.tile([P, KC, 2, P], F32R, tag="eu")
            nc.sync.dma_start(eu[:], euT_d[l].rearrange("h (k p) f -> p k h f", p=P))
            ed = wp.tile([P, 2, D], F32R, tag="ed")
            nc.sync.dma_start(ed[:], edT_d[l].rearrange("h p d -> p h d"))
            bq_c = load_cols(bq_d[l], "bq_c")
            bk_c = load_cols(bk_d[l], "bk_c")
            boe_c = load_cols(boe_d[l], "boe_c")
            l1g = load_cols(ln1g_d[l], "l1g")
            l1b = load_cols(ln1b_d[l], "l1b")
            l2g = load_cols(ln2g_d[l], "l2g")
            l2b = load_cols(ln2b_d[l], "l2b")
            if l == 0:
                emit_bet(0)

            for b in range(NB):
                eT = encT[b]
                qT = xp.tile([P, KC, NT], F32R, tag="qT")
                kT = xp.tile([P, KC, NT], F32R, tag="kT")
                for i in range(KC):
                    pq = big_ps()
                    pk = big_ps()
                    for k in range(KC):
                        nc.tensor.matmul(pq[:], R(wq[:, k, i * P:(i + 1) * P]),
                                         R(eT[:, k, :]), start=(k == 0),
                                         stop=(k == KC - 1))
                        nc.tensor.matmul(pk[:], R(wk[:, k, i * P:(i + 1) * P]),
                                         R(eT[:, k, :]), start=(k == 0),
                                         stop=(k == KC - 1))
                    nc.vector.tensor_scalar(qT[:, i, :], pq[:],
                                            bq_c[:, i:i + 1], None, OP.add)
                    nc.vector.tensor_scalar(kT[:, i, :], pk[:],
                                            bk_c[:, i:i + 1], None, OP.add)
                v_sb = xp.tile([P, KC, D], mybir.dt.bfloat16, tag="v_sb")
                for j in range(KC):
                    pv = big_ps()
                    for k in range(KC):
                        nc.tensor.matmul(pv[:], R(eT[:, k, j * P:(j + 1) * P]),
                                         R(wv[:, k, :]), start=(k == 0),
                                         stop=(k == KC - 1))
                    nc.vector.tensor_copy(v_sb[:, j, :], pv[:])
                oT = xp.tile([P, KC, NT], F32R, tag="oT")
                for g in range(G):
                    exps = [ep.tile([P, KC, NT], mybir.dt.bfloat16, tag=f"expT{i}", name=f"expT{i}")
                            for i in range(4)]
                    for j in range(KC):
                        for i in range(4):
                            psc = big_ps()
                            nc.tensor.matmul(
                                psc[:],
                                R(kT[32 * i:32 * (i + 1), g, j * P:(j + 1) * P]),
                                R(qT[32 * i:32 * (i + 1), g, :]),
                                start=True, stop=True,
                                tile_position=(32 * i, 0))
                            nc.scalar.activation(exps[i][:, j, :], psc[:],
                                                 AF.Exp, scale=SCALE)
                    pden = big_ps()
                    for j in range(KC):
                        for i in range(4):
                            nc.tensor.matmul(pden[32 * i:32 * (i + 1), :],
                                             onesb[:], exps[i][:, j, :],
                                             start=(j == 0), stop=(j == KC - 1),
                                             tile_position=(0, 32 * i))
                    rden = sm2.tile([P, NT], F32, tag="rden")
                    nc.vector.reciprocal(rden[:], pden[:])
                    po = big_ps()
                    for j in range(KC):
                        for i in range(4):
                            h = 4 * g + i
                            nc.tensor.matmul(
                                po[32 * i:32 * (i + 1), :],
                                v_sb[:, j, 32 * h:32 * (h + 1)],
                                exps[i][:, j, :],
                                start=(j == 0), stop=(j == KC - 1),
                                tile_position=(0, 32 * i))
                    nc.vector.tensor_mul(oT[:, g, :], po[:], rden[:])
                s1T = xp.tile([P, KC, NT], F32R, tag="qT")
                for i in range(KC):
                    pa = big_ps()
                    for k in range(KC):
                        nc.tensor.matmul(pa[:], R(wo[:, k, i * P:(i + 1) * P]),
                                         R(oT[:, k, :]), start=(k == 0),
                                         stop=(k == KC - 1))
                    nc.vector.scalar_tensor_tensor(s1T[:, i, :], pa[:],
                                                   boe_c[:, i:i + 1],
                                                   eT[:, i, :], OP.add, OP.add)
                x1T = layernorm_T(s1T, l1g, l1b, "x1T")
                # gate + top-2
                plg = big_ps()
                for k in range(KC):
                    nc.tensor.matmul(plg[:], R(gte[:, k, :]), R(x1T[:, k, :]),
                                     start=(k == 0), stop=(k == KC - 1))
                lg_sb = sm.tile([E, NT], F32, tag="lg_sb")
                nc.vector.tensor_copy(lg_sb[:], plg[0:E, :])
                lgn = sm.tile([P, KC, E], F32, tag="lgn")
                for j in range(KC):
                    pt = small_ps((P, E))
                    nc.tensor.transpose(pt[:], lg_sb[:, j * P:(j + 1) * P],
                                        identf[0:E, 0:E])
                    nc.vector.tensor_copy(lgn[:, j, :], pt[:])
                prb = sm.tile([P, KC, E], F32, tag="prb")
                nc.scalar.activation(prb[:], lgn[:], AF.Exp)
                den = sm.tile([P, KC], F32, tag="gden")
                nc.vector.tensor_reduce(den[:], prb[:], X, OP.add)
                nc.vector.reciprocal(den[:], den[:])
                nc.vector.tensor_tensor(prb[:], prb[:],
                                        den[:, :, None].to_broadcast((P, KC, E)),
                                        OP.mult)
                m1 = sm.tile([P, KC], F32, tag="gm1")
                nc.vector.tensor_reduce(m1[:], prb[:], X, OP.max)
                eq = sm.tile([P, KC, E], F32, tag="geq")
                nc.vector.tensor_tensor(eq[:], prb[:],
                                        m1[:, :, None].to_broadcast((P, KC, E)),
                                        OP.is_ge)
                p2 = sm.tile([P, KC, E], F32, tag="gp2")
                nc.vector.scalar_tensor_tensor(p2[:], eq[:], -2.0, prb[:],
                                               OP.mult, OP.add)
                m2_ = sm.tile([P, KC], F32, tag="gm2")
                nc.vector.tensor_reduce(m2_[:], p2[:], X, OP.max)
                keep = sm.tile([P, KC, E], F32, tag="gkeep")
                nc.vector.tensor_tensor(keep[:], prb[:],
                                        m2_[:, :, None].to_broadcast((P, KC, E)),
                                        OP.is_ge)
                w_nat = sm.tile([P, KC, E], F32, tag="w_nat")
                nc.vector.tensor_mul(w_nat[:], prb[:], keep[:])
                pwT = small_ps((E, NT))
                for j in range(KC):
                    nc.tensor.transpose(pwT[:, j * P:(j + 1) * P],
                                        w_nat[:, j, :], identf[:])
                wT = sm.tile([E, NT], F32R, tag="wT")
                nc.vector.tensor_copy(wT[:], pwT[:])
                wgT = sm.tile([E, NT], F32R, tag="wgT")
                nc.vector.tensor_scalar(wgT[:], wT[:], gamT[l][:], None, OP.mult)
                # experts
                hs = []
                for hh in range(2):
                    pg_ = big_ps()
                    pu_ = big_ps()
                    for k in range(KC):
                        nc.tensor.matmul(pg_[:], R(eg[:, k, hh, :]),
                                         R(x1T[:, k, :]), start=(k == 0),
                                         stop=(k == KC - 1))
                        nc.tensor.matmul(pu_[:], R(eu[:, k, hh, :]),
                                         R(x1T[:, k, :]), start=(k == 0),
                                         stop=(k == KC - 1))
                    pS = big_ps()
                    nc.tensor.matmul(pS[:], R(indp[:, hh * P:(hh + 1) * P]),
                                     R(wgT[:]), start=True, stop=True)
                    en_ = sm2.tile([P, NT], F32, tag="silu_e")
                    nc.scalar.activation(en_[:], pg_[:], AF.Exp, scale=-1.0)
                    nc.vector.tensor_scalar(en_[:], en_[:], 1.0, None, OP.add)
                    nc.vector.reciprocal(en_[:], en_[:])
                    gs = sm2.tile([P, NT], F32, tag="silu_gs")
                    nc.vector.tensor_mul(gs[:], pg_[:], en_[:])
                    hsb = sm2.tile([P, NT], F32R, tag="hsb")
                    nc.vector.tensor_mul(hsb[:], gs[:], pu_[:])
                    nc.vector.tensor_mul(hsb[:], hsb[:], pS[:])
                    hs.append(hsb)
                s2T = xp.tile([P, KC, NT], F32R, tag="kT")
                for i in range(KC):
                    pm_ = big_ps()
                    for hh in range(2):
                        for el in range(4):
                            nc.tensor.matmul(
                                pm_[:],
                                R(ed[32 * el:32 * el + 32, hh,
                                     i * P:(i + 1) * P]),
                                R(hs[hh][32 * el:32 * el + 32, :]),
                                start=(hh == 0 and el == 0), stop=False,
                                tile_position=(32 * el, 0))
                    nc.tensor.matmul(pm_[:], R(betS[l][:, i * P:(i + 1) * P]),
                                     R(wT[:]), start=False, stop=True)
                    nc.vector.tensor_add(s2T[:, i, :], pm_[:], x1T[:, i, :])
                encT[b] = layernorm_T(s2T, l2g, l2b, f"encT{b}")
            if l == 0 and n_layers > 1:
                emit_bet(1)

        # ---- final LN + projection + RevIN denorm ----------------------
        encg_c = load_cols(encg_d, "encg_c")
        encb_c = load_cols(encb_d, "encb_c")
        projT = wp.tile([P, KC, D], F32R, tag="wq")
        nc.sync.dma_start(projT[:], projT_d.rearrange("(k p) o -> p k o", p=P))
        revw_c = row_to_col(revw_r, "revw_c")
        revb_c = row_to_col(revb_r, "revb_c")
        irw_c = cst.tile([P, KC], F32, tag="irw_c")
        nc.vector.tensor_scalar(irw_c[:], revw_c[:], EPS * EPS, None, OP.add)
        nc.vector.reciprocal(irw_c[:], irw_c[:])
        for b in range(NB):
            fT = layernorm_T(encT[b], encg_c, encb_c, "x1T")
            mean_c = row_to_col(mean_rows[b], f"mean_c{b}")
            std_c = row_to_col(std_rows[b], f"std_c{b}")
            s1c = cst.tile([P, KC], F32, tag=f"s1c{b}")
            nc.vector.tensor_mul(s1c[:], irw_c[:], std_c[:])
            for j in range(KC):
                po_ = big_ps()
                for k in range(KC):
                    nc.tensor.matmul(po_[:], R(fT[:, k, j * P:(j + 1) * P]),
                                     R(projT[:, k, :]), start=(k == 0),
                                     stop=False)
                nc.tensor.matmul(po_[:], R(ones[0:2, :]), R(projb_r[0:2, :]),
                                 start=False, stop=True)
                t_ = sm2.tile([P, D], F32, tag="fin_t")
                nc.vector.scalar_tensor_tensor(
                    t_[:], po_[:], revb_c[:, j:j + 1],
                    s1c[:, j:j + 1].to_broadcast((P, D)),
                    OP.subtract, OP.mult)
                ot = sm2.tile([P, D], F32, tag="fin_o")
                nc.vector.tensor_scalar(ot[:], t_[:], mean_c[:, j:j + 1],
                                        None, OP.add)
                nc.sync.dma_start(out_d[b, j * P:(j + 1) * P, :], ot[:])

    nc.compile()
    return nc


_NC_CACHE = {}


def _get_nc():
    if "nc" not in _NC_CACHE:
        _NC_CACHE["nc"] = build_program()
    return _NC_CACHE["nc"]


def _two_row(v):
    out = np.zeros((2, v.shape[0]), np.float32)
    out[0] = v
    return out


def _pad_cols(a):
    out = np.zeros(a.shape[:-1] + (P,), np.float32)
    out[..., :a.shape[-1]] = a
    return out


def pack_inputs(inputs):
    f = np.float32
    Wq, Wk, Wv, Wo = (np.asarray(inputs[k], f) for k in ("Wq", "Wk", "Wv", "Wo"))
    bo, bv = np.asarray(inputs["bo"], f), np.asarray(inputs["bv"], f)
    W_emb = np.asarray(inputs["W_emb"], f)
    exp_gate = np.asarray(inputs["exp_gate"], f)
    exp_up = np.asarray(inputs["exp_up"], f)
    exp_down = np.asarray(inputs["exp_down"], f)
    egT = np.zeros((NL, 2, D, P), f)
    euT = np.zeros((NL, 2, D, P), f)
    edT = np.zeros((NL, 2, P, D), f)
    for l in range(NL):
        for e in range(E):
            h, el = e // 4, e % 4
            egT[l, h, :, 32 * el:32 * el + FF] = exp_gate[l, e].T
            euT[l, h, :, 32 * el:32 * el + FF] = exp_up[l, e].T
            edT[l, h, 32 * el:32 * el + FF, :] = exp_down[l, e].T
    indpad = np.zeros((2, E, P), f)
    for e in range(E):
        indpad[e // 4, e, 32 * (e % 4):32 * (e % 4) + FF] = 1.0
    bo_eff = np.stack([bo[l] + Wo[l] @ bv[l] for l in range(NL)])
    shared = {
        "ins": np.ascontiguousarray(np.asarray(inputs["Ins_tk"], f)[0]),
        "wembT": np.ascontiguousarray(W_emb.T),
        "wsum_r": _two_row(W_emb.sum(1)),
        "wqT": np.ascontiguousarray(np.swapaxes(Wq, 1, 2)),
        "wkT": np.ascontiguousarray(np.swapaxes(Wk, 1, 2)),
        "wvT": np.ascontiguousarray(np.swapaxes(Wv, 1, 2)),
        "woT": np.ascontiguousarray(np.swapaxes(Wo, 1, 2)),
        "bq": np.asarray(inputs["bq"], f), "bk": np.asarray(inputs["bk"], f),
        "bo_eff": np.ascontiguousarray(bo_eff),
        "ln1g": np.asarray(inputs["ln1_g"], f),
        "ln1b": np.asarray(inputs["ln1_b"], f),
        "ln2g": np.asarray(inputs["ln2_g"], f),
        "ln2b": np.asarray(inputs["ln2_b"], f),
        "gateT": _pad_cols(np.swapaxes(np.asarray(inputs["gate_W"], f), 1, 2)),
        "egT": egT, "euT": euT, "edT": edT,
        "gamTw": _pad_cols(np.swapaxes(
            np.asarray(inputs["eilm_gamma_w"], f), 1, 2)),
        "betaT": np.ascontiguousarray(np.swapaxes(
            np.asarray(inputs["eilm_beta_w"], f), 2, 3)),
        "encg": np.asarray(inputs["enc_g"], f),
        "encb": np.asarray(inputs["enc_b"], f),
        "projT": np.ascontiguousarray(np.asarray(inputs["proj_W"], f).T),
        "projb_r": _two_row(np.asarray(inputs["proj_b"], f)),
        "revw_r": _two_row(np.asarray(inputs["rev_w"], f)),
        "revb_r": _two_row(np.asarray(inputs["rev_b"], f)),
        "ident": np.eye(P, dtype=f),
        "identf": np.eye(P, dtype=f),
        "onesm": np.ones((P, P), f),
        "indpad": indpad,
        "zrow": np.zeros((1, D), np.float32),
    }
    return shared


def kernel(**inputs):
    nc = _get_nc()
    shared = pack_inputs(inputs)
    x_seq = np.asarray(inputs["x_seq"], np.float32)
    in_maps = []
    for c in range(8):
        m = dict(shared)
        m["x"] = np.ascontiguousarray(x_seq[NB * c:NB * (c + 1)])
        in_maps.append(m)
    res = run_bass_kernel_spmd(nc, in_maps, core_ids=list(range(8)))
    out = np.concatenate([res.results[c]["out"] for c in range(8)], axis=0)
    return out
